# revision 76
# baseline (speedup 1.0000x reference)
"""Trainium2 Bass kernel for DisentangledSpatialSA.

Reference computation (per batch b, with C=256, IC=128, N=64*64=4096):
    qkv = w_qkv @ x + b_qkv                    # [384, N]
    q, k, v = qkv split into 3 x [IC, N]
    k -= mean_n(k); q -= mean_n(q)             # per-channel spatial centering
    pw[i, j] = sum_c k[c, i] * q[c, j]
    pw = softmax(pw / (sqrt(IC) * TEMP), axis=j)
    y[c, i] = sum_j pw[i, j] * v[c, j]
    out = x + w_out @ y + b_out

Simplifications used (exact up to softmax shift invariance):
  - q centering and the q/k biases cancel inside the row softmax, so only k
    is centered and only v's bias is applied.
  - softmax max-subtraction is skipped: logits are ~N(0, 0.5), safely inside
    fp32 exp range.
  - normalization is applied after the PV matmul: y = (V e) / s, with the
    row sums s computed by a bf16 pairwise tree on VectorE plus one
    gpsimd.partition_all_reduce (which also broadcasts across partitions).

Sharding: data-parallel over batch, one batch element per NeuronCore (8).

Layout: everything channel-major with spatial flattened (n = 4096).
S_t[j, i] tiles are built with keys j on partitions (lhsT = q-tile, rhs = k~),
so the softmax denominators are partition-axis sums; PV uses lhsT = v^T tiles
(DMA transpose of bf16 v) and rhs = exp(S_t).
"""

import numpy as np

import concourse.bacc as bacc
import concourse.bass as bass
import concourse.tile as tile
from concourse import mybir
from concourse import bass_isa
from concourse.bass_utils import run_bass_kernel_spmd
from concourse.masks import make_identity

F32 = mybir.dt.float32
F32R = mybir.dt.float32r
BF16 = mybir.dt.bfloat16

CH = 256
IC = 128
N = 4096
TEMP = 0.05
SCALE = 1.0 / (np.sqrt(np.float32(IC)) * TEMP)  # applied inside exp

P = 128          # partitions
IMW = 1024       # i-macro tile width (query free dim per attention pass)
NMACRO = N // IMW
NJ = N // P      # 32 key tiles
MMF = 512        # max moving free dim per matmul


def build_bass() -> bass.Bass:
    nc = bacc.Bacc("TRN2", target_bir_lowering=False, debug=False, num_devices=8)

    # fp32r-typed external inputs: bits are fp32; fp32r lets matmuls consume
    # them at full (1 cycle/row) rate without an on-chip rounding pass.
    x_d = nc.dram_tensor("x", [CH, N], F32R, kind="ExternalInput")
    wqkvT_d = nc.dram_tensor("wqkvT", [CH, 3 * IC], F32R, kind="ExternalInput")
    bv_d = nc.dram_tensor("bv", [IC, 1], F32, kind="ExternalInput")
    woutT_d = nc.dram_tensor("woutT", [IC, CH], F32R, kind="ExternalInput")
    bout_d = nc.dram_tensor("bout", [CH, 1], F32, kind="ExternalInput")
    out_d = nc.dram_tensor("out", [CH, N], F32, kind="ExternalOutput")

    with tile.TileContext(nc) as tc:
        with (
            tc.tile_pool(name="big", bufs=1) as big,          # long-lived SBUF
            tc.tile_pool(name="small", bufs=1) as small,      # weights/bias
            tc.tile_pool(name="ework", bufs=8) as ework,      # exp tiles
            tc.tile_pool(name="tree", bufs=3) as treep,       # softmax-sum tree
            tc.tile_pool(name="norm", bufs=3) as normp,       # sums/recip
            tc.tile_pool(name="outp", bufs=4) as outp,        # output staging
            tc.tile_pool(name="spsum", bufs=2, space="PSUM") as spsum,  # 4 banks
            tc.tile_pool(name="ypsum", bufs=4, space="PSUM") as ypsum,  # 4 banks
        ):
            # ---------- load inputs ----------
            # Small tensors issue from the Scalar HWDGE queue, bulk x from the
            # Sync queue: dma_start issue costs ~1.3us each, so spreading
            # queues + issuing weights first gets the first matmul going early.
            W = []
            for cchunk in range(2):
                wt = small.tile([P, 3 * IC], F32R, tag=f"w{cchunk}")
                nc.scalar.dma_start(out=wt, in_=wqkvT_d[cchunk * P:(cchunk + 1) * P, :])
                W.append(wt)
            woutT = small.tile([IC, CH], F32R, tag="woutT")
            nc.scalar.dma_start(out=woutT, in_=woutT_d[:, :])
            bv = small.tile([IC, 1], F32, tag="bv")
            nc.scalar.dma_start(out=bv, in_=bv_d[:, :])
            bout_col = []
            for oc in range(2):
                bct = small.tile([P, 1], F32, tag=f"bout{oc}")
                nc.scalar.dma_start(out=bct, in_=bout_d[oc * P:(oc + 1) * P, :])
                bout_col.append(bct)
            ident_bf = small.tile([P, P], BF16, tag="ident")
            make_identity(nc, ident_bf)
            # ~3.4us of dependency-free matmuls: lifts the PE HAM clock gate
            # to 2.4 GHz before the real work lands
            warm_ps = spsum.tile([P, P], F32, tag="s")
            for _ in range(40):
                nc.tensor.matmul(warm_ps, ident_bf, ident_bf, start=True, stop=True)
            X = []
            for cchunk in range(2):
                xt = big.tile([P, N], F32R, tag=f"x{cchunk}")
                for h in range(2):
                    sl = slice(h * (N // 2), (h + 1) * (N // 2))
                    nc.sync.dma_start(
                        out=xt[:, sl], in_=x_d[cchunk * P:(cchunk + 1) * P, sl]
                    )
                X.append(xt)

            # ---------- QKV projection; order chosen so the attention loop's
            # dependencies (k -> centered k, v -> v^T, then q) finish earliest
            q_sb = big.tile([P, N], BF16, tag="q")
            k_bf = big.tile([P, N], BF16, tag="k")
            v_bf = big.tile([P, N], BF16, tag="v")
            vt = big.tile([P, NJ, IC], BF16, tag="vt")

            def qkv_chunk(m, nt, alt_pool=True):
                use_s = alt_pool and nt % 2 == 1
                pool = spsum if use_s else ypsum
                ps = pool.tile([P, MMF], F32, tag="s" if use_s else "ypsum")
                sl = slice(nt * MMF, (nt + 1) * MMF)
                for cchunk in range(2):
                    nc.tensor.matmul(
                        ps,
                        W[cchunk][:, m * IC:(m + 1) * IC],
                        X[cchunk][:, sl],
                        start=(cchunk == 0),
                        stop=(cchunk == 1),
                    )
                if m == 0:
                    with nc.allow_low_precision("q used in bf16 logits"):
                        nc.vector.tensor_copy(q_sb[:, sl], ps)
                elif m == 1:
                    nc.scalar.activation(
                        out=k_bf[:, sl], in_=ps,
                        func=mybir.ActivationFunctionType.Copy,
                    )
                else:
                    with nc.allow_low_precision("v cast to bf16 for PV matmul"):
                        nc.vector.tensor_scalar_add(v_bf[:, sl], ps, bv)
                    # v^T tiles via PE transpose as each chunk lands
                    for jt in range(nt * MMF // P, (nt + 1) * MMF // P):
                        tps = ypsum.tile([P, P], BF16, tag="ypsum")
                        nc.tensor.transpose(
                            tps, v_bf[:, jt * P:(jt + 1) * P], ident_bf
                        )
                        nc.vector.tensor_copy(vt[:, jt, :], tps)

            # q projection fully before attention. k stays UNcentered: the
            # spatial-mean correction enters as a per-partition bias in the
            # exp, bias_j = -scale * (q_j . mean_k). mean_k is derived from x
            # (mk = W_k @ sum_i(x) / N), so the bias block doesn't wait for
            # the k projection at all.
            for nt in range(N // MMF):
                qkv_chunk(0, nt)
            mxr = []
            wkb = small.tile([P, 2, P], BF16, tag="wkb")
            for cchunk in range(2):
                mx = small.tile([P, 1], F32, tag=f"mx{cchunk}")
                nc.vector.tensor_reduce(
                    out=mx, in_=X[cchunk].bitcast(F32),
                    axis=mybir.AxisListType.X, op=mybir.AluOpType.add,
                )
                mxc = small.tile([P, 1], BF16, tag=f"mxr{cchunk}")
                with nc.allow_low_precision("x spatial sum to bf16"):
                    nc.vector.tensor_copy(mxc, mx)
                    nc.vector.tensor_copy(
                        wkb[:, cchunk, :],
                        W[cchunk][:, IC:2 * IC].bitcast(F32),
                    )
                mxr.append(mxc)
            mps = ypsum.tile([P, 1], F32, tag="ypsum")
            for cchunk in range(2):
                nc.tensor.matmul(
                    mps, wkb[:, cchunk, :], mxr[cchunk],
                    start=(cchunk == 0), stop=(cchunk == 1),
                )
            mkr_bf = small.tile([P, 1], BF16, tag="mkr_bf")
            with nc.allow_low_precision("k spatial mean to bf16"):
                nc.vector.tensor_scalar_mul(mkr_bf, mps, 1.0 / N)
            # bias block: one tiny N=1 matmul per key tile, all into one
            # PSUM tile, scaled out with a single DVE op
            bias_all = small.tile([P, NJ], F32, tag="bias_all")
            cps = ypsum.tile([P, NJ], F32, tag="ypsum")
            for jt in range(NJ):
                nc.tensor.matmul(
                    cps[:, jt:jt + 1], q_sb[:, jt * P:(jt + 1) * P], mkr_bf,
                    start=True, stop=True,
                )
            nc.vector.tensor_scalar_mul(bias_all, cps, -float(SCALE))
            # first k chunks: just enough for imacro 0's S matmuls
            qkv_chunk(1, 0)
            qkv_chunk(1, 1)

            # ---------- output projection (emitted per-imacro, interleaved
            # into the NEXT imacro's loop so it never head-of-line blocks PE)
            y_tiles = []
            r_tiles = []

            osb_cur = {}

            def emit_proj_quarter(im, oc, h):
                # y is stored UNnormalized; normalization commutes with the
                # channel-mixing projection, so here:
                #   out = (woutT.T @ y_u) * r + x + bias
                isl = slice(im * IMW, (im + 1) * IMW)
                hsl = slice(h * MMF, (h + 1) * MMF)
                if h == 0:
                    osb_cur[oc] = outp.tile([P, IMW], F32, tag="osb",
                                            name=f"osb{im}_{oc}")
                osb = osb_cur[oc]
                pps = ypsum.tile([P, MMF], F32, tag="ypsum", name=f"pp{im}_{oc}_{h}")
                nc.tensor.matmul(
                    pps,
                    woutT[:, oc * P:(oc + 1) * P],
                    y_tiles[im][:, hsl],
                    start=True,
                    stop=True,
                )
                nc.vector.tensor_mul(osb[:, hsl], pps, r_tiles[im][:, hsl])
                nc.vector.tensor_add(
                    osb[:, hsl], osb[:, hsl],
                    X[oc][:, im * IMW + h * MMF: im * IMW + (h + 1) * MMF].bitcast(F32),
                )
                nc.vector.tensor_scalar_add(osb[:, hsl], osb[:, hsl], bout_col[oc])
                if h == IMW // MMF - 1:
                    nc.sync.dma_start(out=out_d[oc * P:(oc + 1) * P, isl], in_=osb)

            def emit_proj(im):
                for oc in range(2):
                    for h in range(IMW // MMF):
                        emit_proj_quarter(im, oc, h)

            # ---------- attention; imacro 0 also streams in the remaining
            # q/v projection chunks and v^T transposes ----------
            for im in range(NMACRO):
                yhalf = [
                    ypsum.tile([P, MMF], F32, tag="ypsum", name=f"yh{im}_{h}")
                    for h in range(IMW // MMF)
                ]
                levels: list = [None] * 8
                for jt in range(NJ):
                    if im == 0:
                        if jt < N // MMF:
                            qkv_chunk(2, jt, alt_pool=False)  # v chunks
                        elif jt < N // MMF + 6:
                            qkv_chunk(1, jt - (N // MMF) + 2, alt_pool=False)
                    sps = spsum.tile([P, IMW], F32, tag="s")
                    for h in range(IMW // MMF):
                        nc.tensor.matmul(
                            sps[:, h * MMF:(h + 1) * MMF],
                            q_sb[:, jt * P:(jt + 1) * P],
                            k_bf[:, im * IMW + h * MMF: im * IMW + (h + 1) * MMF],
                            start=True,
                            stop=True,
                        )
                    e = ework.tile([P, IMW], BF16, tag="e")
                    nc.scalar.activation(
                        out=e, in_=sps,
                        func=mybir.ActivationFunctionType.Exp,
                        scale=float(SCALE),
                        bias=bias_all[:, jt:jt + 1],
                    )
                    for h in range(IMW // MMF):
                        nc.tensor.matmul(
                            yhalf[h],
                            vt[:, jt, :],
                            e[:, h * MMF:(h + 1) * MMF],
                            start=(jt == 0),
                            stop=(jt == NJ - 1),
                        )
                    # pairwise bf16 tree for the softmax denominators
                    cur, lvl = e, 0
                    with nc.allow_low_precision("softmax denom tree in bf16"):
                        while levels[lvl] is not None:
                            nxt = treep.tile([P, IMW], BF16, tag=f"tree{lvl}")
                            nc.vector.tensor_add(nxt, levels[lvl], cur)
                            levels[lvl] = None
                            cur = nxt
                            lvl += 1
                    levels[lvl] = cur
                total = levels[5]
                assert total is not None
                # copy unnormalized y out first: releases the PSUM
                # accumulators immediately, so the next imacro's PV matmuls
                # never wait on the reduce/reciprocal chain below
                y_sb = big.tile([P, IMW], F32R, tag=f"ysb{im}")
                for h in range(IMW // MMF):
                    nc.vector.tensor_copy(
                        y_sb[:, h * MMF:(h + 1) * MMF], yhalf[h]
                    )
                y_tiles.append(y_sb)
                # sum over in-tile j (partition axis), broadcast to all rows
                s_bc = normp.tile([P, IMW], F32, tag="sbc")
                r_bc = big.tile([P, IMW], F32, tag=f"rbc{im}")
                r_scr = normp.tile([P, IMW], F32, tag="rscr")
                nc.gpsimd.partition_all_reduce(
                    s_bc, total, channels=P, reduce_op=bass_isa.ReduceOp.add
                )
                nc.vector.reciprocal_approx_accurate(r_bc, s_bc, scratch=r_scr)
                r_tiles.append(r_bc)
            # keep the PE clock warm across the final partition_all_reduce so
            # the last projection matmuls don't run throttled
            warm_ps2 = spsum.tile([P, P], F32, tag="s")
            for _ in range(64):
                nc.tensor.matmul(warm_ps2, ident_bf, ident_bf, start=True, stop=True)
            for im in range(NMACRO):
                emit_proj(im)
    nc.compile()
    return nc


_CACHED_NC = None


def _get_nc():
    global _CACHED_NC
    if _CACHED_NC is None:
        _CACHED_NC = build_bass()
    return _CACHED_NC


def _prep_in_maps(x, w_qkv, b_qkv, w_out, b_out):
    xs = np.ascontiguousarray(np.asarray(x, np.float32).reshape(8, CH, N))
    wqkvT = np.ascontiguousarray(np.asarray(w_qkv, np.float32).T)
    bv = np.ascontiguousarray(
        np.asarray(b_qkv, np.float32)[2 * IC:3 * IC].reshape(IC, 1)
    )
    woutT = np.ascontiguousarray(np.asarray(w_out, np.float32).T)
    bout = np.ascontiguousarray(np.asarray(b_out, np.float32).reshape(CH, 1))
    return [
        {
            "x": np.ascontiguousarray(xs[i]),
            "wqkvT": wqkvT,
            "bv": bv,
            "woutT": woutT,
            "bout": bout,
        }
        for i in range(8)
    ]


def kernel(x, w_qkv, b_qkv, w_out, b_out, _trace=False, _trace_kwargs=None):
    nc = _get_nc()
    in_maps = _prep_in_maps(x, w_qkv, b_qkv, w_out, b_out)
    res = run_bass_kernel_spmd(
        nc, in_maps, core_ids=list(range(8)), trace=_trace,
        **(_trace_kwargs or {}),
    )
    out = np.stack([res.results[i]["out"] for i in range(8)])
    out = out.reshape(8, CH, 64, 64).astype(np.float32)
    if _trace:
        return out, res
    return out


if __name__ == "__main__":
    rng = np.random.default_rng(0)
    x = rng.standard_normal((8, CH, 64, 64), dtype=np.float32)
    w_qkv = (rng.standard_normal((3 * IC, CH), dtype=np.float32) * 0.01)
    b_qkv = (rng.standard_normal((3 * IC,), dtype=np.float32) * 0.01)
    w_out = (rng.standard_normal((CH, IC), dtype=np.float32) * 0.01)
    b_out = (rng.standard_normal((CH,), dtype=np.float32) * 0.01)
    o = kernel(x, w_qkv=w_qkv, b_qkv=b_qkv, w_out=w_out, b_out=b_out)
    print(o.shape, o.dtype)


# revision 77
# speedup vs baseline: 1.0241x; 1.0241x over previous
"""Trainium2 Bass kernel for DisentangledSpatialSA.

Reference computation (per batch b, with C=256, IC=128, N=64*64=4096):
    qkv = w_qkv @ x + b_qkv                    # [384, N]
    q, k, v = qkv split into 3 x [IC, N]
    k -= mean_n(k); q -= mean_n(q)             # per-channel spatial centering
    pw[i, j] = sum_c k[c, i] * q[c, j]
    pw = softmax(pw / (sqrt(IC) * TEMP), axis=j)
    y[c, i] = sum_j pw[i, j] * v[c, j]
    out = x + w_out @ y + b_out

Simplifications used (exact up to softmax shift invariance):
  - q centering and the q/k biases cancel inside the row softmax, so only k
    is centered and only v's bias is applied.
  - softmax max-subtraction is skipped: logits are ~N(0, 0.5), safely inside
    fp32 exp range.
  - normalization is applied after the PV matmul: y = (V e) / s, with the
    row sums s computed by a bf16 pairwise tree on VectorE plus one
    gpsimd.partition_all_reduce (which also broadcasts across partitions).

Sharding: data-parallel over batch, one batch element per NeuronCore (8).

Layout: everything channel-major with spatial flattened (n = 4096).
S_t[j, i] tiles are built with keys j on partitions (lhsT = q-tile, rhs = k~),
so the softmax denominators are partition-axis sums; PV uses lhsT = v^T tiles
(DMA transpose of bf16 v) and rhs = exp(S_t).
"""

import numpy as np

import concourse.bacc as bacc
import concourse.bass as bass
import concourse.tile as tile
from concourse import mybir
from concourse import bass_isa
from concourse.bass_utils import run_bass_kernel_spmd
from concourse.masks import make_identity

F32 = mybir.dt.float32
F32R = mybir.dt.float32r
BF16 = mybir.dt.bfloat16

CH = 256
IC = 128
N = 4096
TEMP = 0.05
SCALE = 1.0 / (np.sqrt(np.float32(IC)) * TEMP)  # applied inside exp

P = 128          # partitions
IMW = 1024       # i-macro tile width (query free dim per attention pass)
NMACRO = N // IMW
NJ = N // P      # 32 key tiles
MMF = 512        # max moving free dim per matmul


def build_bass() -> bass.Bass:
    nc = bacc.Bacc("TRN2", target_bir_lowering=False, debug=False, num_devices=8)

    # fp32r-typed external inputs: bits are fp32; fp32r lets matmuls consume
    # them at full (1 cycle/row) rate without an on-chip rounding pass.
    x_d = nc.dram_tensor("x", [CH, N], F32R, kind="ExternalInput")
    wqkvT_d = nc.dram_tensor("wqkvT", [CH, 3 * IC], F32R, kind="ExternalInput")
    bv_d = nc.dram_tensor("bv", [IC, 1], F32, kind="ExternalInput")
    woutT_d = nc.dram_tensor("woutT", [IC, CH], F32R, kind="ExternalInput")
    bout_d = nc.dram_tensor("bout", [CH, 1], F32, kind="ExternalInput")
    out_d = nc.dram_tensor("out", [CH, N], F32, kind="ExternalOutput")

    with tile.TileContext(nc) as tc:
        with (
            tc.tile_pool(name="big", bufs=1) as big,          # long-lived SBUF
            tc.tile_pool(name="small", bufs=1) as small,      # weights/bias
            tc.tile_pool(name="ework", bufs=8) as ework,      # exp tiles
            tc.tile_pool(name="tree", bufs=3) as treep,       # softmax-sum tree
            tc.tile_pool(name="norm", bufs=3) as normp,       # sums/recip
            tc.tile_pool(name="outp", bufs=4) as outp,        # output staging
            tc.tile_pool(name="spsum", bufs=2, space="PSUM") as spsum,  # 4 banks
            tc.tile_pool(name="ypsum", bufs=4, space="PSUM") as ypsum,  # 4 banks
        ):
            # ---------- load inputs ----------
            # Small tensors issue from the Scalar HWDGE queue, bulk x from the
            # Sync queue: dma_start issue costs ~1.3us each, so spreading
            # queues + issuing weights first gets the first matmul going early.
            W = []
            for cchunk in range(2):
                wt = small.tile([P, 3 * IC], F32R, tag=f"w{cchunk}")
                nc.scalar.dma_start(out=wt, in_=wqkvT_d[cchunk * P:(cchunk + 1) * P, :])
                W.append(wt)
            woutT = small.tile([IC, CH], F32R, tag="woutT")
            nc.scalar.dma_start(out=woutT, in_=woutT_d[:, :])
            bv = small.tile([IC, 1], F32, tag="bv")
            nc.scalar.dma_start(out=bv, in_=bv_d[:, :])
            bout_col = []
            for oc in range(2):
                bct = small.tile([P, 1], F32, tag=f"bout{oc}")
                nc.scalar.dma_start(out=bct, in_=bout_d[oc * P:(oc + 1) * P, :])
                bout_col.append(bct)
            ident_bf = small.tile([P, P], BF16, tag="ident")
            make_identity(nc, ident_bf)
            # ~3.4us of dependency-free matmuls: lifts the PE HAM clock gate
            # to 2.4 GHz before the real work lands
            warm_ps = spsum.tile([P, P], F32, tag="s")
            for _ in range(40):
                nc.tensor.matmul(warm_ps, ident_bf, ident_bf, start=True, stop=True)
            X = []
            for cchunk in range(2):
                xt = big.tile([P, N], F32R, tag=f"x{cchunk}")
                for h in range(2):
                    sl = slice(h * (N // 2), (h + 1) * (N // 2))
                    nc.sync.dma_start(
                        out=xt[:, sl], in_=x_d[cchunk * P:(cchunk + 1) * P, sl]
                    )
                X.append(xt)

            # ---------- QKV projection; order chosen so the attention loop's
            # dependencies (k -> centered k, v -> v^T, then q) finish earliest
            q_sb = big.tile([P, N], BF16, tag="q")
            k_bf = big.tile([P, N], BF16, tag="k")
            v_bf = big.tile([P, N], BF16, tag="v")
            vt = big.tile([P, NJ, IC], BF16, tag="vt")

            def qkv_chunk(m, nt, alt_pool=True):
                use_s = alt_pool and nt % 2 == 1
                pool = spsum if use_s else ypsum
                ps = pool.tile([P, MMF], F32, tag="s" if use_s else "ypsum")
                sl = slice(nt * MMF, (nt + 1) * MMF)
                for cchunk in range(2):
                    nc.tensor.matmul(
                        ps,
                        W[cchunk][:, m * IC:(m + 1) * IC],
                        X[cchunk][:, sl],
                        start=(cchunk == 0),
                        stop=(cchunk == 1),
                    )
                if m == 0:
                    with nc.allow_low_precision("q used in bf16 logits"):
                        nc.vector.tensor_copy(q_sb[:, sl], ps)
                elif m == 1:
                    nc.scalar.activation(
                        out=k_bf[:, sl], in_=ps,
                        func=mybir.ActivationFunctionType.Copy,
                    )
                else:
                    with nc.allow_low_precision("v cast to bf16 for PV matmul"):
                        nc.vector.tensor_scalar_add(v_bf[:, sl], ps, bv)
                    # v^T tiles via PE transpose as each chunk lands
                    for jt in range(nt * MMF // P, (nt + 1) * MMF // P):
                        tps = ypsum.tile([P, P], BF16, tag="ypsum")
                        nc.tensor.transpose(
                            tps, v_bf[:, jt * P:(jt + 1) * P], ident_bf
                        )
                        nc.vector.tensor_copy(vt[:, jt, :], tps)

            # q projection fully before attention. k stays UNcentered: the
            # spatial-mean correction enters as a per-partition bias in the
            # exp, bias_j = -scale * (q_j . mean_k). mean_k is derived from x
            # (mk = W_k @ sum_i(x) / N), so the bias block doesn't wait for
            # the k projection at all.
            for nt in range(N // MMF):
                qkv_chunk(0, nt)
            mxr = []
            wkb = small.tile([P, 2, P], BF16, tag="wkb")
            for cchunk in range(2):
                mx = small.tile([P, 1], F32, tag=f"mx{cchunk}")
                nc.vector.tensor_reduce(
                    out=mx, in_=X[cchunk].bitcast(F32),
                    axis=mybir.AxisListType.X, op=mybir.AluOpType.add,
                )
                mxc = small.tile([P, 1], BF16, tag=f"mxr{cchunk}")
                with nc.allow_low_precision("x spatial sum to bf16"):
                    nc.vector.tensor_copy(mxc, mx)
                    nc.vector.tensor_copy(
                        wkb[:, cchunk, :],
                        W[cchunk][:, IC:2 * IC].bitcast(F32),
                    )
                mxr.append(mxc)
            mps = ypsum.tile([P, 1], F32, tag="ypsum")
            for cchunk in range(2):
                nc.tensor.matmul(
                    mps, wkb[:, cchunk, :], mxr[cchunk],
                    start=(cchunk == 0), stop=(cchunk == 1),
                )
            mkr_bf = small.tile([P, 1], BF16, tag="mkr_bf")
            with nc.allow_low_precision("k spatial mean to bf16"):
                nc.vector.tensor_scalar_mul(mkr_bf, mps, 1.0 / N)
            # bias block: one tiny N=1 matmul per key tile, all into one
            # PSUM tile, scaled out with a single DVE op
            bias_all = small.tile([P, NJ], F32, tag="bias_all")
            cps = ypsum.tile([P, NJ], F32, tag="ypsum")
            for jt in range(NJ):
                nc.tensor.matmul(
                    cps[:, jt:jt + 1], q_sb[:, jt * P:(jt + 1) * P], mkr_bf,
                    start=True, stop=True,
                )
            nc.vector.tensor_scalar_mul(bias_all, cps, -float(SCALE))
            # first k chunks: just enough for imacro 0's S matmuls
            qkv_chunk(1, 0)
            qkv_chunk(1, 1)

            # ---------- output projection (emitted per-imacro, interleaved
            # into the NEXT imacro's loop so it never head-of-line blocks PE)
            y_tiles = []
            r_tiles = []

            osb_cur = {}

            def emit_proj_quarter(im, oc, h):
                # y is stored UNnormalized; normalization commutes with the
                # channel-mixing projection, so here:
                #   out = (woutT.T @ y_u) * r + x + bias
                isl = slice(im * IMW, (im + 1) * IMW)
                hsl = slice(h * MMF, (h + 1) * MMF)
                if h == 0:
                    osb_cur[oc] = outp.tile([P, IMW], F32, tag="osb",
                                            name=f"osb{im}_{oc}")
                osb = osb_cur[oc]
                pps = ypsum.tile([P, MMF], F32, tag="ypsum", name=f"pp{im}_{oc}_{h}")
                nc.tensor.matmul(
                    pps,
                    woutT[:, oc * P:(oc + 1) * P],
                    y_tiles[im][:, hsl],
                    start=True,
                    stop=True,
                )
                nc.vector.tensor_mul(osb[:, hsl], pps, r_tiles[im][:, hsl])
                nc.vector.tensor_add(
                    osb[:, hsl], osb[:, hsl],
                    X[oc][:, im * IMW + h * MMF: im * IMW + (h + 1) * MMF].bitcast(F32),
                )
                # bias on the (tail-idle) ScalarE, in place
                nc.scalar.activation(
                    out=osb[:, hsl], in_=osb[:, hsl],
                    func=mybir.ActivationFunctionType.Identity,
                    bias=bout_col[oc], scale=1.0,
                )
                if h == IMW // MMF - 1:
                    nc.sync.dma_start(out=out_d[oc * P:(oc + 1) * P, isl], in_=osb)

            def emit_proj(im):
                for oc in range(2):
                    for h in range(IMW // MMF):
                        emit_proj_quarter(im, oc, h)

            # ---------- attention; imacro 0 also streams in the remaining
            # q/v projection chunks and v^T transposes ----------
            for im in range(NMACRO):
                yhalf = [
                    ypsum.tile([P, MMF], F32, tag="ypsum", name=f"yh{im}_{h}")
                    for h in range(IMW // MMF)
                ]
                levels: list = [None] * 8
                for jt in range(NJ):
                    if im == 0:
                        if jt < N // MMF:
                            qkv_chunk(2, jt, alt_pool=False)  # v chunks
                        elif jt < N // MMF + 6:
                            qkv_chunk(1, jt - (N // MMF) + 2, alt_pool=False)
                    sps = spsum.tile([P, IMW], F32, tag="s")
                    for h in range(IMW // MMF):
                        nc.tensor.matmul(
                            sps[:, h * MMF:(h + 1) * MMF],
                            q_sb[:, jt * P:(jt + 1) * P],
                            k_bf[:, im * IMW + h * MMF: im * IMW + (h + 1) * MMF],
                            start=True,
                            stop=True,
                        )
                    e = ework.tile([P, IMW], BF16, tag="e")
                    nc.scalar.activation(
                        out=e, in_=sps,
                        func=mybir.ActivationFunctionType.Exp,
                        scale=float(SCALE),
                        bias=bias_all[:, jt:jt + 1],
                    )
                    for h in range(IMW // MMF):
                        nc.tensor.matmul(
                            yhalf[h],
                            vt[:, jt, :],
                            e[:, h * MMF:(h + 1) * MMF],
                            start=(jt == 0),
                            stop=(jt == NJ - 1),
                        )
                    # pairwise bf16 tree for the softmax denominators
                    cur, lvl = e, 0
                    with nc.allow_low_precision("softmax denom tree in bf16"):
                        while levels[lvl] is not None:
                            nxt = treep.tile([P, IMW], BF16, tag=f"tree{lvl}")
                            nc.vector.tensor_add(nxt, levels[lvl], cur)
                            levels[lvl] = None
                            cur = nxt
                            lvl += 1
                    levels[lvl] = cur
                total = levels[5]
                assert total is not None
                # copy unnormalized y out first: releases the PSUM
                # accumulators immediately, so the next imacro's PV matmuls
                # never wait on the reduce/reciprocal chain below
                y_sb = big.tile([P, IMW], F32R, tag=f"ysb{im}")
                for h in range(IMW // MMF):
                    nc.vector.tensor_copy(
                        y_sb[:, h * MMF:(h + 1) * MMF], yhalf[h]
                    )
                y_tiles.append(y_sb)
                # sum over in-tile j (partition axis), broadcast to all rows
                s_bc = normp.tile([P, IMW], F32, tag="sbc")
                r_bc = big.tile([P, IMW], F32, tag=f"rbc{im}")
                r_scr = normp.tile([P, IMW], F32, tag="rscr")
                nc.gpsimd.partition_all_reduce(
                    s_bc, total, channels=P, reduce_op=bass_isa.ReduceOp.add
                )
                nc.vector.reciprocal_approx_accurate(r_bc, s_bc, scratch=r_scr)
                r_tiles.append(r_bc)
            # keep the PE clock warm across the final partition_all_reduce so
            # the last projection matmuls don't run throttled
            warm_ps2 = spsum.tile([P, P], F32, tag="s")
            for _ in range(64):
                nc.tensor.matmul(warm_ps2, ident_bf, ident_bf, start=True, stop=True)
            for im in range(NMACRO):
                emit_proj(im)
    nc.compile()
    return nc


_CACHED_NC = None


def _get_nc():
    global _CACHED_NC
    if _CACHED_NC is None:
        _CACHED_NC = build_bass()
    return _CACHED_NC


def _prep_in_maps(x, w_qkv, b_qkv, w_out, b_out):
    xs = np.ascontiguousarray(np.asarray(x, np.float32).reshape(8, CH, N))
    wqkvT = np.ascontiguousarray(np.asarray(w_qkv, np.float32).T)
    bv = np.ascontiguousarray(
        np.asarray(b_qkv, np.float32)[2 * IC:3 * IC].reshape(IC, 1)
    )
    woutT = np.ascontiguousarray(np.asarray(w_out, np.float32).T)
    bout = np.ascontiguousarray(np.asarray(b_out, np.float32).reshape(CH, 1))
    return [
        {
            "x": np.ascontiguousarray(xs[i]),
            "wqkvT": wqkvT,
            "bv": bv,
            "woutT": woutT,
            "bout": bout,
        }
        for i in range(8)
    ]


def kernel(x, w_qkv, b_qkv, w_out, b_out, _trace=False, _trace_kwargs=None):
    nc = _get_nc()
    in_maps = _prep_in_maps(x, w_qkv, b_qkv, w_out, b_out)
    res = run_bass_kernel_spmd(
        nc, in_maps, core_ids=list(range(8)), trace=_trace,
        **(_trace_kwargs or {}),
    )
    out = np.stack([res.results[i]["out"] for i in range(8)])
    out = out.reshape(8, CH, 64, 64).astype(np.float32)
    if _trace:
        return out, res
    return out


if __name__ == "__main__":
    rng = np.random.default_rng(0)
    x = rng.standard_normal((8, CH, 64, 64), dtype=np.float32)
    w_qkv = (rng.standard_normal((3 * IC, CH), dtype=np.float32) * 0.01)
    b_qkv = (rng.standard_normal((3 * IC,), dtype=np.float32) * 0.01)
    w_out = (rng.standard_normal((CH, IC), dtype=np.float32) * 0.01)
    b_out = (rng.standard_normal((CH,), dtype=np.float32) * 0.01)
    o = kernel(x, w_qkv=w_qkv, b_qkv=b_qkv, w_out=w_out, b_out=b_out)
    print(o.shape, o.dtype)


# revision 79
# speedup vs baseline: 1.0242x; 1.0000x over previous
"""Trainium2 Bass kernel for DisentangledSpatialSA.

Reference computation (per batch b, with C=256, IC=128, N=64*64=4096):
    qkv = w_qkv @ x + b_qkv                    # [384, N]
    q, k, v = qkv split into 3 x [IC, N]
    k -= mean_n(k); q -= mean_n(q)             # per-channel spatial centering
    pw[i, j] = sum_c k[c, i] * q[c, j]
    pw = softmax(pw / (sqrt(IC) * TEMP), axis=j)
    y[c, i] = sum_j pw[i, j] * v[c, j]
    out = x + w_out @ y + b_out

Simplifications used (exact up to softmax shift invariance):
  - q centering and the q/k biases cancel inside the row softmax, so only k
    is centered and only v's bias is applied.
  - softmax max-subtraction is skipped: logits are ~N(0, 0.5), safely inside
    fp32 exp range.
  - normalization is applied after the PV matmul: y = (V e) / s, with the
    row sums s computed by a bf16 pairwise tree on VectorE plus one
    gpsimd.partition_all_reduce (which also broadcasts across partitions).

Sharding: data-parallel over batch, one batch element per NeuronCore (8).

Layout: everything channel-major with spatial flattened (n = 4096).
S_t[j, i] tiles are built with keys j on partitions (lhsT = q-tile, rhs = k~),
so the softmax denominators are partition-axis sums; PV uses lhsT = v^T tiles
(DMA transpose of bf16 v) and rhs = exp(S_t).
"""

import numpy as np

import concourse.bacc as bacc
import concourse.bass as bass
import concourse.tile as tile
from concourse import mybir
from concourse import bass_isa
from concourse.bass_utils import run_bass_kernel_spmd
from concourse.masks import make_identity

F32 = mybir.dt.float32
F32R = mybir.dt.float32r
BF16 = mybir.dt.bfloat16

CH = 256
IC = 128
N = 4096
TEMP = 0.05
SCALE = 1.0 / (np.sqrt(np.float32(IC)) * TEMP)  # applied inside exp

P = 128          # partitions
IMW = 1024       # i-macro tile width (query free dim per attention pass)
NMACRO = N // IMW
NJ = N // P      # 32 key tiles
MMF = 512        # max moving free dim per matmul


def build_bass() -> bass.Bass:
    nc = bacc.Bacc("TRN2", target_bir_lowering=False, debug=False, num_devices=8)

    # fp32r-typed external inputs: bits are fp32; fp32r lets matmuls consume
    # them at full (1 cycle/row) rate without an on-chip rounding pass.
    x_d = nc.dram_tensor("x", [CH, N], F32R, kind="ExternalInput")
    wqkvT_d = nc.dram_tensor("wqkvT", [CH, 3 * IC], F32R, kind="ExternalInput")
    bv_d = nc.dram_tensor("bv", [IC, 1], F32, kind="ExternalInput")
    woutT_d = nc.dram_tensor("woutT", [IC, CH], F32R, kind="ExternalInput")
    bout_d = nc.dram_tensor("bout", [CH, 1], F32, kind="ExternalInput")
    out_d = nc.dram_tensor("out", [CH, N], F32, kind="ExternalOutput")

    with tile.TileContext(nc) as tc:
        with (
            tc.tile_pool(name="big", bufs=1) as big,          # long-lived SBUF
            tc.tile_pool(name="small", bufs=1) as small,      # weights/bias
            tc.tile_pool(name="ework", bufs=10) as ework,     # exp tiles
            tc.tile_pool(name="tree", bufs=3) as treep,       # softmax-sum tree
            tc.tile_pool(name="norm", bufs=3) as normp,       # sums/recip
            tc.tile_pool(name="outp", bufs=4) as outp,        # output staging
            tc.tile_pool(name="spsum", bufs=2, space="PSUM") as spsum,  # 4 banks
            tc.tile_pool(name="ypsum", bufs=4, space="PSUM") as ypsum,  # 4 banks
        ):
            # ---------- load inputs ----------
            # Small tensors issue from the Scalar HWDGE queue, bulk x from the
            # Sync queue: dma_start issue costs ~1.3us each, so spreading
            # queues + issuing weights first gets the first matmul going early.
            W = []
            for cchunk in range(2):
                wt = small.tile([P, 3 * IC], F32R, tag=f"w{cchunk}")
                nc.scalar.dma_start(out=wt, in_=wqkvT_d[cchunk * P:(cchunk + 1) * P, :])
                W.append(wt)
            woutT = small.tile([IC, CH], F32R, tag="woutT")
            nc.scalar.dma_start(out=woutT, in_=woutT_d[:, :])
            bv = small.tile([IC, 1], F32, tag="bv")
            nc.scalar.dma_start(out=bv, in_=bv_d[:, :])
            bout_col = []
            for oc in range(2):
                bct = small.tile([P, 1], F32, tag=f"bout{oc}")
                nc.scalar.dma_start(out=bct, in_=bout_d[oc * P:(oc + 1) * P, :])
                bout_col.append(bct)
            ident_bf = small.tile([P, P], BF16, tag="ident")
            make_identity(nc, ident_bf)
            # ~3.4us of dependency-free matmuls: lifts the PE HAM clock gate
            # to 2.4 GHz before the real work lands
            warm_ps = spsum.tile([P, P], F32, tag="s")
            for _ in range(40):
                nc.tensor.matmul(warm_ps, ident_bf, ident_bf, start=True, stop=True)
            X = []
            for cchunk in range(2):
                xt = big.tile([P, N], F32R, tag=f"x{cchunk}")
                for h in range(2):
                    sl = slice(h * (N // 2), (h + 1) * (N // 2))
                    nc.sync.dma_start(
                        out=xt[:, sl], in_=x_d[cchunk * P:(cchunk + 1) * P, sl]
                    )
                X.append(xt)

            # ---------- QKV projection; order chosen so the attention loop's
            # dependencies (k -> centered k, v -> v^T, then q) finish earliest
            q_sb = big.tile([P, N], BF16, tag="q")
            k_bf = big.tile([P, N], BF16, tag="k")
            v_bf = big.tile([P, N], BF16, tag="v")
            vt = big.tile([P, NJ, IC], BF16, tag="vt")

            def qkv_chunk(m, nt, alt_pool=True):
                use_s = alt_pool and nt % 2 == 1
                pool = spsum if use_s else ypsum
                ps = pool.tile([P, MMF], F32, tag="s" if use_s else "ypsum")
                sl = slice(nt * MMF, (nt + 1) * MMF)
                for cchunk in range(2):
                    nc.tensor.matmul(
                        ps,
                        W[cchunk][:, m * IC:(m + 1) * IC],
                        X[cchunk][:, sl],
                        start=(cchunk == 0),
                        stop=(cchunk == 1),
                    )
                if m == 0:
                    with nc.allow_low_precision("q used in bf16 logits"):
                        nc.vector.tensor_copy(q_sb[:, sl], ps)
                elif m == 1:
                    nc.scalar.activation(
                        out=k_bf[:, sl], in_=ps,
                        func=mybir.ActivationFunctionType.Copy,
                    )
                else:
                    with nc.allow_low_precision("v cast to bf16 for PV matmul"):
                        nc.vector.tensor_scalar_add(v_bf[:, sl], ps, bv)
                    # v^T tiles via PE transpose as each chunk lands
                    for jt in range(nt * MMF // P, (nt + 1) * MMF // P):
                        tps = ypsum.tile([P, P], BF16, tag="ypsum")
                        nc.tensor.transpose(
                            tps, v_bf[:, jt * P:(jt + 1) * P], ident_bf
                        )
                        nc.vector.tensor_copy(vt[:, jt, :], tps)

            # q projection fully before attention. k stays UNcentered: the
            # spatial-mean correction enters as a per-partition bias in the
            # exp, bias_j = -scale * (q_j . mean_k). mean_k is derived from x
            # (mk = W_k @ sum_i(x) / N), so the bias block doesn't wait for
            # the k projection at all.
            for nt in range(N // MMF):
                qkv_chunk(0, nt)
            mxr = []
            wkb = small.tile([P, 2, P], BF16, tag="wkb")
            for cchunk in range(2):
                mx = small.tile([P, 1], F32, tag=f"mx{cchunk}")
                nc.vector.tensor_reduce(
                    out=mx, in_=X[cchunk].bitcast(F32),
                    axis=mybir.AxisListType.X, op=mybir.AluOpType.add,
                )
                mxc = small.tile([P, 1], BF16, tag=f"mxr{cchunk}")
                with nc.allow_low_precision("x spatial sum to bf16"):
                    nc.vector.tensor_copy(mxc, mx)
                    nc.vector.tensor_copy(
                        wkb[:, cchunk, :],
                        W[cchunk][:, IC:2 * IC].bitcast(F32),
                    )
                mxr.append(mxc)
            mps = ypsum.tile([P, 1], F32, tag="ypsum")
            for cchunk in range(2):
                nc.tensor.matmul(
                    mps, wkb[:, cchunk, :], mxr[cchunk],
                    start=(cchunk == 0), stop=(cchunk == 1),
                )
            mkr_bf = small.tile([P, 1], BF16, tag="mkr_bf")
            with nc.allow_low_precision("k spatial mean to bf16"):
                nc.vector.tensor_scalar_mul(mkr_bf, mps, 1.0 / N)
            # bias block: one tiny N=1 matmul per key tile, all into one
            # PSUM tile, scaled out with a single DVE op
            bias_all = small.tile([P, NJ], F32, tag="bias_all")
            cps = ypsum.tile([P, NJ], F32, tag="ypsum")
            for jt in range(NJ):
                nc.tensor.matmul(
                    cps[:, jt:jt + 1], q_sb[:, jt * P:(jt + 1) * P], mkr_bf,
                    start=True, stop=True,
                )
                if jt % 8 == 7:
                    # chunked write: exp(0) only waits for the first 8 biases
                    nc.vector.tensor_scalar_mul(
                        bias_all[:, jt - 7:jt + 1], cps[:, jt - 7:jt + 1],
                        -float(SCALE),
                    )
            # first k chunks: just enough for imacro 0's S matmuls
            qkv_chunk(1, 0)
            qkv_chunk(1, 1)

            # ---------- output projection (emitted per-imacro, interleaved
            # into the NEXT imacro's loop so it never head-of-line blocks PE)
            y_tiles = []
            r_tiles = []

            osb_cur = {}

            def emit_proj_quarter(im, oc, h):
                # y is stored UNnormalized; normalization commutes with the
                # channel-mixing projection, so here:
                #   out = (woutT.T @ y_u) * r + x + bias
                isl = slice(im * IMW, (im + 1) * IMW)
                hsl = slice(h * MMF, (h + 1) * MMF)
                if h == 0:
                    osb_cur[oc] = outp.tile([P, IMW], F32, tag="osb",
                                            name=f"osb{im}_{oc}")
                osb = osb_cur[oc]
                pps = ypsum.tile([P, MMF], F32, tag="ypsum", name=f"pp{im}_{oc}_{h}")
                nc.tensor.matmul(
                    pps,
                    woutT[:, oc * P:(oc + 1) * P],
                    y_tiles[im][:, hsl],
                    start=True,
                    stop=True,
                )
                nc.vector.tensor_mul(osb[:, hsl], pps, r_tiles[im][:, hsl])
                nc.vector.tensor_add(
                    osb[:, hsl], osb[:, hsl],
                    X[oc][:, im * IMW + h * MMF: im * IMW + (h + 1) * MMF].bitcast(F32),
                )
                # bias on the (tail-idle) ScalarE, in place
                nc.scalar.activation(
                    out=osb[:, hsl], in_=osb[:, hsl],
                    func=mybir.ActivationFunctionType.Identity,
                    bias=bout_col[oc], scale=1.0,
                )
                if h == IMW // MMF - 1:
                    nc.sync.dma_start(out=out_d[oc * P:(oc + 1) * P, isl], in_=osb)

            def emit_proj(im):
                for oc in range(2):
                    for h in range(IMW // MMF):
                        emit_proj_quarter(im, oc, h)

            # ---------- attention; imacro 0 also streams in the remaining
            # q/v projection chunks and v^T transposes ----------
            for im in range(NMACRO):
                yhalf = [
                    ypsum.tile([P, MMF], F32, tag="ypsum", name=f"yh{im}_{h}")
                    for h in range(IMW // MMF)
                ]
                levels: list = [None] * 8
                for jt in range(NJ):
                    if im == 0:
                        if jt < N // MMF:
                            qkv_chunk(2, jt, alt_pool=False)  # v chunks
                        elif jt < N // MMF + 6:
                            qkv_chunk(1, jt - (N // MMF) + 2, alt_pool=False)
                    sps = spsum.tile([P, IMW], F32, tag="s")
                    for h in range(IMW // MMF):
                        nc.tensor.matmul(
                            sps[:, h * MMF:(h + 1) * MMF],
                            q_sb[:, jt * P:(jt + 1) * P],
                            k_bf[:, im * IMW + h * MMF: im * IMW + (h + 1) * MMF],
                            start=True,
                            stop=True,
                        )
                    e = ework.tile([P, IMW], BF16, tag="e")
                    nc.scalar.activation(
                        out=e, in_=sps,
                        func=mybir.ActivationFunctionType.Exp,
                        scale=float(SCALE),
                        bias=bias_all[:, jt:jt + 1],
                    )
                    for h in range(IMW // MMF):
                        nc.tensor.matmul(
                            yhalf[h],
                            vt[:, jt, :],
                            e[:, h * MMF:(h + 1) * MMF],
                            start=(jt == 0),
                            stop=(jt == NJ - 1),
                        )
                    # pairwise bf16 tree for the softmax denominators
                    cur, lvl = e, 0
                    with nc.allow_low_precision("softmax denom tree in bf16"):
                        while levels[lvl] is not None:
                            nxt = treep.tile([P, IMW], BF16, tag=f"tree{lvl}")
                            nc.vector.tensor_add(nxt, levels[lvl], cur)
                            levels[lvl] = None
                            cur = nxt
                            lvl += 1
                    levels[lvl] = cur
                total = levels[5]
                assert total is not None
                # copy unnormalized y out first: releases the PSUM
                # accumulators immediately, so the next imacro's PV matmuls
                # never wait on the reduce/reciprocal chain below
                y_sb = big.tile([P, IMW], F32R, tag=f"ysb{im}")
                for h in range(IMW // MMF):
                    nc.vector.tensor_copy(
                        y_sb[:, h * MMF:(h + 1) * MMF], yhalf[h]
                    )
                y_tiles.append(y_sb)
                # sum over in-tile j (partition axis), broadcast to all rows
                s_bc = normp.tile([P, IMW], F32, tag="sbc")
                r_bc = big.tile([P, IMW], F32, tag=f"rbc{im}")
                r_scr = normp.tile([P, IMW], F32, tag="rscr")
                nc.gpsimd.partition_all_reduce(
                    s_bc, total, channels=P, reduce_op=bass_isa.ReduceOp.add
                )
                nc.vector.reciprocal_approx_accurate(r_bc, s_bc, scratch=r_scr)
                r_tiles.append(r_bc)
            # keep the PE clock warm across the final partition_all_reduce so
            # the last projection matmuls don't run throttled
            warm_ps2 = spsum.tile([P, P], F32, tag="s")
            for _ in range(64):
                nc.tensor.matmul(warm_ps2, ident_bf, ident_bf, start=True, stop=True)
            for im in range(NMACRO):
                emit_proj(im)
    nc.compile()
    return nc


_CACHED_NC = None


def _get_nc():
    global _CACHED_NC
    if _CACHED_NC is None:
        _CACHED_NC = build_bass()
    return _CACHED_NC


def _prep_in_maps(x, w_qkv, b_qkv, w_out, b_out):
    xs = np.ascontiguousarray(np.asarray(x, np.float32).reshape(8, CH, N))
    wqkvT = np.ascontiguousarray(np.asarray(w_qkv, np.float32).T)
    bv = np.ascontiguousarray(
        np.asarray(b_qkv, np.float32)[2 * IC:3 * IC].reshape(IC, 1)
    )
    woutT = np.ascontiguousarray(np.asarray(w_out, np.float32).T)
    bout = np.ascontiguousarray(np.asarray(b_out, np.float32).reshape(CH, 1))
    return [
        {
            "x": np.ascontiguousarray(xs[i]),
            "wqkvT": wqkvT,
            "bv": bv,
            "woutT": woutT,
            "bout": bout,
        }
        for i in range(8)
    ]


def kernel(x, w_qkv, b_qkv, w_out, b_out, _trace=False, _trace_kwargs=None):
    nc = _get_nc()
    in_maps = _prep_in_maps(x, w_qkv, b_qkv, w_out, b_out)
    res = run_bass_kernel_spmd(
        nc, in_maps, core_ids=list(range(8)), trace=_trace,
        **(_trace_kwargs or {}),
    )
    out = np.stack([res.results[i]["out"] for i in range(8)])
    out = out.reshape(8, CH, 64, 64).astype(np.float32)
    if _trace:
        return out, res
    return out


if __name__ == "__main__":
    rng = np.random.default_rng(0)
    x = rng.standard_normal((8, CH, 64, 64), dtype=np.float32)
    w_qkv = (rng.standard_normal((3 * IC, CH), dtype=np.float32) * 0.01)
    b_qkv = (rng.standard_normal((3 * IC,), dtype=np.float32) * 0.01)
    w_out = (rng.standard_normal((CH, IC), dtype=np.float32) * 0.01)
    b_out = (rng.standard_normal((CH,), dtype=np.float32) * 0.01)
    o = kernel(x, w_qkv=w_qkv, b_qkv=b_qkv, w_out=w_out, b_out=b_out)
    print(o.shape, o.dtype)
